# revision 1
# baseline (speedup 1.0000x reference)
"""Trainium2 Bass kernel for the MLPSim adjacency-constructor problem.

Full shapes: spatial [4, 2048, 32], temporal [4, 288, 32], output
adj [4, 2336, 2336] f32 where adj = tanh(relu(blocks)):
  ss = tanh(m - m^T), m = nv1 @ nv2^T, nv_i = tanh(3*x@W_i^T)
  st = s1[n] + s2[t] + b_st ;  ts = s1t[t] + s2t[n] + b_ts
  tt = triu(temporal @ temporal^T)

Sharding: 8 cores = (batch b = c//2) x (row-half h = c%2); each core emits
1024 spatial + 144 temporal rows ([1168, 2336], ~10.9 MB) of one batch.

Raw-bass implementation (hand sync): the installed walrus rejects any
instruction carrying more than one inline sync-wait (Tile-generated
kernels all do), so every wait here is a standalone wait_ge instruction.
Engines run sequential programs synchronized by five monotone semaphores;
psum/sbuf buffers ping-pong with distance-2/3 reuse guarded by waits.

Device algebra: tanh(relu(x)) == relu(tanh(x)) -> both tanh passes on ACT,
one relu pass on DVE at 2x mode; ss rows as ONE K=64 matmul via
L = [nv1^T_rows ; -nv2^T_rows], R = [nv2^T_all ; nv1^T_all];
st/ts via K=1 ones-matmul broadcast of s2 plus per-partition ACT bias s1.
"""

import numpy as np
from contextlib import ExitStack

import concourse.bass as bass
from concourse import mybir
from concourse.bass_utils import run_bass_kernel_spmd

AF = mybir.ActivationFunctionType
F32 = mybir.dt.float32

B, N, T, D = 4, 2048, 288, 32
NS = N // 2
TS = T // 2
NT = N + T
ROWS = NS + TS
N_CORES = 8
NCHUNK = NS // 128   # 8 spatial row-chunks


def build_program():
    nc = bass.Bass()
    inp = {}

    def di(name, shape):
        inp[name] = nc.declare_dram_parameter(name, list(shape), F32, isOutput=False)

    for name, shape in (
        ("spT_all", (D, N)), ("spT_rows", (D, NS)), ("tmT_all", (D, T)),
        ("tmT_rows", (D, TS)), ("W12T", (D, 2 * D)), ("wst_a", (D, 1)),
        ("wst_b", (D, 1)), ("wts_a", (D, 1)), ("wts_b", (D, 1)),
        ("bst", (1, 1)), ("bts", (1, 1)), ("ttmask", (TS, T)),
    ):
        di(name, shape)
    out = nc.declare_dram_parameter("out", [ROWS, NT], F32, isOutput=True)

    ctx = ExitStack()
    _uid = [0]

    def sbuf(shape):
        _uid[0] += 1
        return ctx.enter_context(nc.sbuf_tensor(f"sb{_uid[0]}", shape, F32))

    def psum(shape):
        _uid[0] += 1
        return ctx.enter_context(nc.psum_tensor(f"ps{_uid[0]}", shape, F32))

    with ctx:
        t_in = {k: sbuf(list(v.shape)) for k, v in inp.items() if k != "ttmask"}
        mask0 = sbuf([128, T])
        mask1 = sbuf([TS - 128, T])
        Lt = sbuf([2 * D, NS])
        Rt = sbuf([2 * D, N])
        ones = sbuf([1, 128])
        s1col = sbuf([128, NCHUNK])
        s2row = sbuf([1, T])
        s1tcol = sbuf([128, 2])
        s2trow = sbuf([1, N])
        t1bufs = [sbuf([128, N]) for _ in range(2)]
        prebufs = [sbuf([128, NT]) for _ in range(3)]
        outbufs = [sbuf([128, NT]) for _ in range(3)]
        tttbuf = sbuf([128, T])

        zps = [psum([128, 1024]), psum([128, 1024])]
        pps = [psum([128, 512]), psum([128, 512])]
        qps = [psum([1, 512]), psum([1, 512])]

        dmain = ctx.enter_context(nc.semaphore("dmain"))
        pe_s = ctx.enter_context(nc.semaphore("pe_s"))
        act_s = ctx.enter_context(nc.semaphore("act_s"))
        dve_s = ctx.enter_context(nc.semaphore("dve_s"))
        douts = [ctx.enter_context(nc.semaphore(f"dout{k}")) for k in range(3)]
        SEM = {"pe": pe_s, "act": act_s, "dve": dve_s, "din": dmain,
               "dout0": douts[0], "dout1": douts[1], "dout2": douts[2]}

        # plan[engine] = list of (waits, fn, inc_sem_name)
        plan = {"sync": [], "tensor": [], "scalar": [], "vector": []}
        cnt = {"pe": 0, "act": 0, "dve": 0, "din": 0,
               "dout0": 0, "dout1": 0, "dout2": 0}

        def op(engine, waits, fn, inc=None, delta=None):
            plan[engine].append((waits or [], fn, inc))
            if inc:
                if delta is None:
                    delta = 16 if inc.startswith("d") and inc != "dve" else 1
                cnt[inc] += delta
                return cnt[inc]
            return None

        # ---------- input loads ----------
        for name, tt in t_in.items():
            op("sync", None, lambda t=tt, s=inp[name]: nc.sync.dma_start(out=t[:], in_=s[:]), "din", delta=16)
        op("sync", None, lambda: nc.sync.dma_start(out=mask0[:], in_=inp["ttmask"][0:128, :]), "din", delta=16)
        op("sync", None, lambda: nc.sync.dma_start(out=mask1[:], in_=inp["ttmask"][128:TS, :]), "din", delta=16)
        din_all = cnt["din"]

        W12T = t_in["W12T"]
        mm = nc.tensor.matmul
        act_i = nc.scalar.activation

        def pe(waits, fn, inc=None):
            return op("tensor", waits, fn, inc)

        def act(waits, fn):
            return op("scalar", waits, fn, "act")

        def dve(waits, fn):
            return op("vector", waits, fn, "dve")

        # ---------- nv prep ----------
        def mm2(dst, lhsT_ap, rhs_t, c0, waits, rows=2 * D):
            pe(waits, lambda: mm(dst[0:rows, 0:512], lhsT_ap, rhs_t[:, c0:c0 + 512],
                                 start=True, stop=True))
            return pe(None, lambda: mm(dst[0:rows, 512:1024], lhsT_ap,
                                       rhs_t[:, c0 + 512:c0 + 1024],
                                       start=True, stop=True), "pe")

        g1 = mm2(zps[0], W12T[:], t_in["spT_all"], 0, [("din", din_all)])
        g2 = mm2(zps[1], W12T[:], t_in["spT_all"], 1024, None)
        a_z0 = act([("pe", g1)], lambda: act_i(Rt[D:2 * D, 0:1024], zps[0][0:D, :], AF.Tanh, scale=3.0))
        a_z0 = act(None, lambda: act_i(Rt[0:D, 0:1024], zps[0][D:2 * D, :], AF.Tanh, scale=3.0))
        act([("pe", g2)], lambda: act_i(Rt[D:2 * D, 1024:2048], zps[1][0:D, :], AF.Tanh, scale=3.0))
        a_z1 = act(None, lambda: act_i(Rt[0:D, 1024:2048], zps[1][D:2 * D, :], AF.Tanh, scale=3.0))
        g3 = mm2(zps[0], W12T[:], t_in["spT_rows"], 0, [("act", a_z0)])
        act([("pe", g3)], lambda: act_i(Lt[0:D, :], zps[0][0:D, :], AF.Tanh, scale=3.0))
        a_L = act(None, lambda: act_i(Lt[D:2 * D, :], zps[0][D:2 * D, :], AF.Tanh, scale=-3.0))

        # ---------- small vectors ----------
        dve(None, lambda: nc.vector.memset(ones[:], 1.0))
        for i in range(NCHUNK):
            g_s1 = pe(None, lambda i=i: mm(pps[0][:, i:i + 1],
                                           t_in["spT_rows"][:, i * 128:(i + 1) * 128],
                                           t_in["wst_a"][:], start=True, stop=True),
                      "pe" if i == NCHUNK - 1 else None)
        g_sv = None
        pe(None, lambda: mm(pps[1][0:1, 0:T], t_in["wst_b"][:], t_in["tmT_all"][:],
                            start=True, stop=True))
        pe(None, lambda: mm(pps[1][:, 300:301], t_in["tmT_rows"][:, 0:128],
                            t_in["wts_a"][:], start=True, stop=True))
        g_sv = pe(None, lambda: mm(pps[1][0:TS - 128, 301:302], t_in["tmT_rows"][:, 128:TS],
                                   t_in["wts_a"][:], start=True, stop=True), "pe")

        d1 = dve([("pe", g_s1)], lambda: nc.vector.tensor_copy(s1col[:], pps[0][:, 0:NCHUNK]))
        dve([("pe", g_sv)], lambda: nc.vector.tensor_scalar_add(s2row[:], pps[1][0:1, 0:T],
                                                                t_in["bst"][0:1, 0:1]))
        dve(None, lambda: nc.vector.tensor_copy(s1tcol[:, 0:1], pps[1][:, 300:301]))
        dve(None, lambda: nc.vector.tensor_copy(s1tcol[0:TS - 128, 1:2], pps[1][0:TS - 128, 301:302]))
        d_add = []
        qg = []
        for j in range(4):
            w = [("dve", d_add[j - 2])] if j >= 2 else None
            qg.append(pe(w, lambda j=j: mm(qps[j % 2][:], t_in["wts_b"][:],
                                           t_in["spT_all"][:, j * 512:(j + 1) * 512],
                                           start=True, stop=True), "pe"))
            d_add.append(dve([("pe", qg[j])],
                             lambda j=j: nc.vector.tensor_scalar_add(
                                 s2trow[0:1, j * 512:(j + 1) * 512], qps[j % 2][:],
                                 t_in["bts"][0:1, 0:1])))
        d_sv = d_add[-1]

        # ---------- main loop ----------
        zact = []     # act value after the z-consuming ACT of z-step s
        pez = []      # pe value after z matmuls of z-step s
        st_a2 = []    # act value after pre fully written (per spatial chunk)
        relu_d = []   # dve value after relu of out-chunk r
        outdma = []   # dout value after store of out-chunk r

        def zstep(s, lhs_ap, rhs_t, c0, rows, extra):
            waits = list(extra or [])
            if s >= 2:
                waits.append(("act", zact[s - 2]))
            pe(waits, lambda: mm(zps[s % 2][0:rows, 0:512], lhs_ap, rhs_t[:, c0:c0 + 512],
                                 start=True, stop=True))
            g = pe(None, lambda: mm(zps[s % 2][0:rows, 512:1024], lhs_ap,
                                    rhs_t[:, c0 + 512:c0 + 1024], start=True, stop=True), "pe")
            pez.append(g)

        s = 0
        for i in range(NCHUNK):
            rs = slice(i * 128, (i + 1) * 128)
            t1 = t1bufs[i % 2]
            pre = prebufs[i % 3]
            for j in range(2):
                zstep(s, Lt[:, rs], Rt, j * 1024, 128, [("act", a_L)] if s < 2 else None)
                zact.append(act([("pe", pez[s])],
                                lambda t1=t1, j=j, s=s: act_i(t1[:, j * 1024:(j + 1) * 1024],
                                                              zps[s % 2][:], AF.Tanh)))
                s += 1
            stw = [("act", st_a2[i - 2])] if i >= 2 else [("dve", d_sv)]
            gst = pe(stw, lambda i=i: mm(pps[i % 2][:, 0:T], ones[:], s2row[:],
                                         start=True, stop=True), "pe")
            ow = ([("dve", relu_d[i - 3])] if i >= 3 else []) + [("act", zact[s - 1])]
            act(ow, lambda pre=pre, t1=t1: act_i(pre[:, 0:N], t1[:], AF.Tanh))
            a2 = act([("pe", gst)], lambda pre=pre, i=i: act_i(pre[:, N:NT], pps[i % 2][:, 0:T],
                                                               AF.Tanh, bias=s1col[:, i:i + 1]))
            st_a2.append(a2)
            ob = outbufs[i % 3]
            rw = [("act", a2)] + ([(f"dout{i % 3}", outdma[i - 3])] if i >= 3 else [])
            relu_d.append(dve(rw, lambda ob=ob, pre=pre: nc.vector.tensor_scalar_max(
                ob[:], pre[:], 0.0)))
            outdma.append(op("sync", [("dve", relu_d[i])],
                             lambda ob=ob, rs=rs: nc.sync.dma_start(out=out[rs, :], in_=ob[:]),
                             f"dout{i % 3}", delta=16))

        # temporal chunks
        a_tt_prev = None
        for k, (t0, tn) in enumerate(((0, 128), (128, TS - 128))):
            r = NCHUNK + k
            pre = prebufs[r % 3]
            ow = [("dve", relu_d[r - 3])]
            for j in range(2):
                zstep(s, ones[:, 0:tn], s2trow, j * 1024, tn, [("dve", d_sv)])
                zact.append(act([("pe", pez[s])] + (ow if j == 0 else []),
                                lambda pre=pre, j=j, s=s, tn=tn, k=k: act_i(
                                    pre[0:tn, j * 1024:(j + 1) * 1024], zps[s % 2][0:tn, :],
                                    AF.Tanh, bias=s1tcol[0:tn, k:k + 1])))
                s += 1
            gtw = [("act", st_a2[NCHUNK - 2 + k])]
            gtt = pe(gtw, lambda t0=t0, tn=tn, k=k: mm(pps[k % 2][0:tn, 0:T],
                                                       t_in["tmT_rows"][:, t0:t0 + tn],
                                                       t_in["tmT_all"][:], start=True, stop=True),
                     "pe")
            attw = [("pe", gtt)] + ([("dve", a_tt_prev)] if a_tt_prev else [])
            att = act(attw, lambda tn=tn, k=k: act_i(tttbuf[0:tn, :], pps[k % 2][0:tn, 0:T],
                                                     AF.Tanh))
            mt = mask0 if k == 0 else mask1
            a_tt_prev = dve([("act", att)], lambda pre=pre, tn=tn, mt=mt: nc.vector.tensor_mul(
                pre[0:tn, N:NT], tttbuf[0:tn, :], mt[:]))
            ob = outbufs[r % 3]
            rw = [("dve", a_tt_prev), (f"dout{r % 3}", outdma[r - 3])]
            relu_d.append(dve(rw, lambda ob=ob, pre=pre, tn=tn: nc.vector.tensor_scalar_max(
                ob[0:tn, :], pre[0:tn, :], 0.0)))
            outdma.append(op("sync", [("dve", relu_d[r])],
                             lambda ob=ob, t0=t0, tn=tn: nc.sync.dma_start(
                                 out=out[NS + t0:NS + t0 + tn, :], in_=ob[0:tn, :]),
                             f"dout{r % 3}", delta=16))

        # ---------- emit ----------
        with nc.Block() as block:
            def make_body(engine_name):
                ops = plan[engine_name]

                def body(eng):
                    satisfied = {}
                    for waits, fn, inc in ops:
                        for sem_name, val in waits:
                            if val is not None and satisfied.get(sem_name, -1) < val:
                                eng.wait_ge(SEM[sem_name], val)
                                satisfied[sem_name] = val
                        ins = fn()
                        if inc is None:
                            continue
                        if inc == "din" or inc.startswith("dout"):
                            ins.then_inc(SEM[inc], 16)
                        else:
                            ins.then_inc(SEM[inc], 1)
                return body

            block.sync(make_body("sync"))
            block.tensor(make_body("tensor"))
            block.scalar(make_body("scalar"))
            block.vector(make_body("vector"))

    return nc


def build_in_maps(spatial_nodes, temporal_nodes, W_ss1, W_ss2, w_st, b_st, w_ts, b_ts):
    f = np.float32
    W12T = np.ascontiguousarray(np.concatenate([W_ss1.T, W_ss2.T], axis=1), dtype=f)
    in_maps = []
    for c in range(N_CORES):
        b, h = divmod(c, 2)
        tmask = (np.arange(T)[None, :] >= (h * TS + np.arange(TS))[:, None]).astype(f)
        in_maps.append({
            "spT_all": np.ascontiguousarray(spatial_nodes[b].T, dtype=f),
            "spT_rows": np.ascontiguousarray(spatial_nodes[b, h * NS:(h + 1) * NS].T, dtype=f),
            "tmT_all": np.ascontiguousarray(temporal_nodes[b].T, dtype=f),
            "tmT_rows": np.ascontiguousarray(temporal_nodes[b, h * TS:(h + 1) * TS].T, dtype=f),
            "W12T": W12T,
            "wst_a": np.ascontiguousarray(w_st[:D, None], dtype=f),
            "wst_b": np.ascontiguousarray(w_st[D:, None], dtype=f),
            "wts_a": np.ascontiguousarray(w_ts[:D, None], dtype=f),
            "wts_b": np.ascontiguousarray(w_ts[D:, None], dtype=f),
            "bst": np.asarray(b_st, dtype=f).reshape(1, 1),
            "bts": np.asarray(b_ts, dtype=f).reshape(1, 1),
            "ttmask": tmask,
        })
    return in_maps


def assemble(results):
    out = np.empty((B, NT, NT), np.float32)
    for c in range(N_CORES):
        b, h = divmod(c, 2)
        r = results[c]["out"]
        out[b, h * NS:(h + 1) * NS, :] = r[0:NS]
        out[b, N + h * TS: N + (h + 1) * TS, :] = r[NS:ROWS]
    return out


_NC = None


def kernel(**inputs):
    global _NC
    if _NC is None:
        _NC = build_program()
    in_maps = build_in_maps(**inputs)
    res = run_bass_kernel_spmd(_NC, in_maps, list(range(N_CORES)))
    return assemble(res.results)



# revision 15
# speedup vs baseline: 1.1299x; 1.1299x over previous
"""Trainium2 Bass kernel for the MLPSim adjacency-constructor problem.

Full shapes: spatial [4, 2048, 32], temporal [4, 288, 32], output
adj [4, 2336, 2336] f32 where adj = tanh(relu(blocks)):
  ss = tanh(m - m^T), m = nv1 @ nv2^T, nv_i = tanh(3*x@W_i^T)
  st = s1[n] + s2[t] + b_st ;  ts = s1t[t] + s2t[n] + b_ts
  tt = triu(temporal @ temporal^T)

Sharding: 8 cores = (batch b = c//2) x (row-half h = c%2); each core emits
1024 spatial + 144 temporal rows ([1168, 2336]) of one batch, stored fp16
on device and upcast to f32 on the host during unshard.

Key device algebra (ACT is the wall: 1 elem/lane/cycle, dtype-independent):
  ss out = tanh(tanh(relu(z))), z = m - m^T. A minimax fit
    tanh(tanh(relu(z))) ~= (C + D*yr)*yr,  yr = max(tanh(A*z), 0)
  with A=1.15118303 C=0.90136458 D=-0.141975 (max err 2.2e-3) collapses the
  two full-size ACT tanh passes into ONE (scale A folded into the ACT op);
  the quadratic runs on DVE in 3 fused 16-bit ops (4x/4x/2x modes).
  z itself is an fp16 matmul with an hi/lo split (z = [Lh;Lh]@[Rh;Rl] +
  Ll@Rh, fp16 at 1 cyc/col vs f32 4 cyc/col) so matmul rounding stays ~1e-4.
  st is one K=9 fp16 matmul (8 chunk-indicator rows + s2 row) + one ACT
  pass for all chunks; ts is K=1 rank-1 fp16 matmuls; tt stays f32.
"""

import numpy as np
from contextlib import ExitStack

import concourse.bass as bass
from concourse import mybir
from concourse.bass_utils import run_bass_kernel_spmd

AF = mybir.ActivationFunctionType
ALU = mybir.AluOpType
F32 = mybir.dt.float32
F16 = mybir.dt.float16

B, N, T, D = 4, 2048, 288, 32
NS = N // 2          # spatial rows per core
TS = T // 2          # temporal rows per core
NT = N + T
ROWS = NS + TS
N_CORES = 8
NCHUNK = NS // 128   # 8 spatial row-chunks

# minimax fit of tanh(tanh(relu(z))) ~= (CA + DA*yr)*yr, yr = max(tanh(AA*z),0)
AA = 1.15118303
CA = 0.90136458
DA = -0.141975


def build_program():
    nc = bass.Bass()
    inp = {}

    for name, shape, dt in (
        ("spT32", (D, N), F32), ("spT16", (D, N), F16),
        ("spTr32", (D, NS), F32), ("spTr16", (D, NS), F16),
        ("tmT32", (D, T), F32), ("tmT16", (D, T), F16),
        ("tmTr32", (D, TS), F32), ("tmTr16", (D, TS), F16),
        ("W12T_R", (D, 2 * D), F32), ("W12T_L", (D, 2 * D), F32),
        ("wst_a16", (D, 1), F16), ("wst_b16", (D, 1), F16),
        ("wts_a16", (D, 1), F16), ("wts_b16", (D, 1), F16),
        ("bst", (1, 1), F32), ("bts", (1, 1), F32),
        ("ttmask16", (TS, T), F16),
        ("stind", (9, NCHUNK * T), F16),
    ):
        inp[name] = nc.declare_dram_parameter(name, list(shape), dt, isOutput=False)
    out = nc.declare_dram_parameter("out", [ROWS, NT], F16, isOutput=True)

    ctx = ExitStack()
    _uid = [0]

    def sbuf(shape, dt=F16):
        _uid[0] += 1
        return ctx.enter_context(nc.sbuf_tensor(f"sb{_uid[0]}", list(shape), dt))

    with ctx:
        t_in = {k: sbuf(v.shape, v.dtype) for k, v in inp.items() if k != "ttmask16"}
        masks = [sbuf([128, T]), sbuf([TS - 128, T])]
        Rf32 = sbuf([2 * D, N], F32)
        Lf32 = sbuf([2 * D, NS], F32)
        Rhl = sbuf([128, N])          # [Rh(64) ; Rl(64)] fp16
        Rlo = sbuf([2 * D, N])        # staging for the partition shift
        Lhh = sbuf([128, NS])         # [Lh ; Lh] fp16
        Llo = sbuf([2 * D, NS])
        s1row = sbuf([1, NS])
        s2row = sbuf([1, T])
        s1t = sbuf([1, TS])
        s2tb = sbuf([1, N])
        ones = sbuf([1, N])
        stL = sbuf([9, 128])
        y_st = sbuf([128, NCHUNK * T])
        ybufs = [sbuf([128, N]) for _ in range(3)]
        yrb = sbuf([128, N])
        wb = sbuf([128, N])
        ytb = sbuf([128, N])
        tttb = sbuf([128, T])
        outbufs = [sbuf([128, NT]) for _ in range(3)]

        dmain = ctx.enter_context(nc.semaphore("dmain"))
        dmx_s = ctx.enter_context(nc.semaphore("dmx"))
        pe_s = ctx.enter_context(nc.semaphore("pe_s"))
        act_s = ctx.enter_context(nc.semaphore("act_s"))
        dve_s = ctx.enter_context(nc.semaphore("dve_s"))
        douts = [ctx.enter_context(nc.semaphore(f"dout{k}")) for k in range(3)]
        SEM = {"pe": pe_s, "act": act_s, "dve": dve_s, "din": dmain, "dmx": dmx_s,
               "dout0": douts[0], "dout1": douts[1], "dout2": douts[2]}

        plan = {"sync": [], "tensor": [], "scalar": [], "vector": []}
        cnt = {k: 0 for k in SEM}

        def op(engine, waits, fn, inc=None):
            plan[engine].append((waits or [], fn, inc))
            if inc:
                cnt[inc] += 16 if (inc.startswith("d") and inc != "dve") else 1
                return cnt[inc]
            return None

        def pe(waits, fn, inc=None):
            return op("tensor", waits, fn, inc)

        def act(waits, fn):
            return op("scalar", waits, fn, "act")

        def dve(waits, fn):
            return op("vector", waits, fn, "dve")

        mm = nc.tensor.matmul
        act_i = nc.scalar.activation
        V = nc.vector

        # ---------- input loads ----------
        for name, tt_ in t_in.items():
            op("sync", None, lambda t=tt_, s=inp[name]: nc.sync.dma_start(out=t[:], in_=s[:]), "din")
        op("sync", None, lambda: nc.sync.dma_start(out=masks[0][:],
                                                   in_=inp["ttmask16"][0:128, :]), "din")
        op("sync", None, lambda: nc.sync.dma_start(out=masks[1][:],
                                                   in_=inp["ttmask16"][128:TS, :]), "din")
        din_all = cnt["din"]
        W = [("din", din_all)]

        # ================= PREP PHASE A: pu [64,2048] + sv [1,2048] =========
        with nc.psum_tensor("pu", [2 * D, N], F32) as pu, \
             nc.psum_tensor("sv", [1, N], F32) as sv:
            # nv pre-acts for R = [nv2; nv1] over all N nodes (f32, exact)
            for c in range(4):
                g_pu = pe(W if c == 0 else None,
                          lambda c=c: mm(pu[:, c * 512:(c + 1) * 512], t_in["W12T_R"][:],
                                         t_in["spT32"][:, c * 512:(c + 1) * 512],
                                         start=True, stop=True), "pe" if c == 3 else None)
            # svec round 1: s1 (rows-half), s2, s1t  (fp16 matvecs as psum rows)
            pe(None, lambda: mm(sv[0:1, 0:512], t_in["wst_a16"][:],
                                t_in["spTr16"][:, 0:512], start=True, stop=True))
            pe(None, lambda: mm(sv[0:1, 512:1024], t_in["wst_a16"][:],
                                t_in["spTr16"][:, 512:1024], start=True, stop=True))
            pe(None, lambda: mm(sv[0:1, 1024:1024 + T], t_in["wst_b16"][:],
                                t_in["tmT16"][:], start=True, stop=True))
            g_sv1 = pe(None, lambda: mm(sv[0:1, 1312:1312 + TS], t_in["wts_a16"][:],
                                        t_in["tmTr16"][:], start=True, stop=True), "pe")

            a_Rf = act([("pe", g_pu)], lambda: act_i(Rf32[:], pu[:], AF.Tanh, scale=3.0))

            d_s1 = dve([("pe", g_sv1)], lambda: V.tensor_copy(s1row[:], sv[0:1, 0:NS]))
            d_s2 = dve(None, lambda: V.tensor_scalar_add(s2row[:], sv[0:1, 1024:1024 + T],
                                                         t_in["bst"][0:1, 0:1]))
            d_s1t = dve(None, lambda: V.tensor_copy(s1t[:], sv[0:1, 1312:1312 + TS]))

            # svec round 2: s2t over all N (overwrites sv)
            for c in range(4):
                g_sv2 = pe([("dve", d_s1t)] if c == 0 else None,
                           lambda c=c: mm(sv[0:1, c * 512:(c + 1) * 512], t_in["wts_b16"][:],
                                          t_in["spT16"][:, c * 512:(c + 1) * 512],
                                          start=True, stop=True), "pe" if c == 3 else None)
            d_s2t = dve([("pe", g_sv2)], lambda: V.tensor_scalar_add(s2tb[:], sv[0:1, :],
                                                                     t_in["bts"][0:1, 0:1]))

        # DVE: hi/lo split of R; ones rows
        stR = t_in["stind"]   # row 0 blank (s2 DMA'd in), rows 1..8 = indicators
        d_Rh = dve([("act", a_Rf)], lambda: V.tensor_copy(Rhl[0:2 * D, :], Rf32[:]))
        d_Rlo = dve(None, lambda: V.tensor_sub(Rlo[:], Rf32[:], Rhl[0:2 * D, :]))
        dve(None, lambda: V.memset(ones[:], 1.0))
        dve(None, lambda: V.memset(stL[0:1, :], 1.0))

        # aux DMAs: partition shifts / reshapes (sbuf->sbuf)
        x_Rlo = op("sync", [("dve", d_Rlo)],
                   lambda: nc.sync.dma_start(out=Rhl[2 * D:128, :], in_=Rlo[:]), "dmx")
        x_s1 = op("sync", [("dve", d_s1)],
                  lambda: nc.sync.dma_start(out=stL[1:9, :], in_=s1row[:]), "dmx")
        for k in range(NCHUNK):
            x_s2 = op("sync", [("dve", d_s2)],
                      lambda k=k: nc.sync.dma_start(out=stR[0:1, k * T:(k + 1) * T],
                                                    in_=s2row[:]), "dmx")

        # ================= PREP PHASE B: pv [64,1024] + stp [128,2304] ======
        with nc.psum_tensor("pv", [2 * D, NS], F32) as pv, \
             nc.psum_tensor("stp", [128, NCHUNK * T], F32) as stp:
            # nv pre-acts for L = [nv1; -nv2] over rows-half (banks freed by a_Rf)
            for c in range(2):
                g_pv = pe([("act", a_Rf)] if c == 0 else None,
                          lambda c=c: mm(pv[:, c * 512:(c + 1) * 512], t_in["W12T_L"][:],
                                         t_in["spTr32"][:, c * 512:(c + 1) * 512],
                                         start=True, stop=True), "pe" if c == 1 else None)
            a_Lf = act([("pe", g_pv)], lambda: act_i(Lf32[:], pv[:], AF.Tanh, scale=3.0))
            d_Lh = dve([("act", a_Lf)], lambda: V.tensor_copy(Lhh[0:2 * D, :], Lf32[:]))
            d_Llo = dve(None, lambda: V.tensor_sub(Llo[:], Lf32[:], Lhh[0:2 * D, :]))
            x_Lh = op("sync", [("dve", d_Lh)],
                      lambda: nc.sync.dma_start(out=Lhh[2 * D:128, :], in_=Lhh[0:2 * D, :]),
                      "dmx")

            # st block: K=9 fp16 matmul -> one ACT pass for all 8 chunks
            stw = [("dmx", cnt["dmx"]), ("dve", d_s2t), ("act", a_Rf)]
            npc = NCHUNK * T  # 2304
            for c in range(5):
                c0, c1 = c * 512, min((c + 1) * 512, npc)
                g_stp = pe(stw if c == 0 else None,
                           lambda c0=c0, c1=c1: mm(stp[:, c0:c1], stL[:],
                                                   stR[:, c0:c1], start=True, stop=True),
                           "pe" if c == 4 else None)
            a_yst = act([("pe", g_stp)], lambda: act_i(y_st[:], stp[:], AF.Tanh))

        # ================= MAIN: zps 2x [128, 2048] =========================
        with nc.psum_tensor("zA", [128, N], F32) as zA, \
             nc.psum_tensor("zB", [128, N], F32) as zB:
            zps = [zA, zB]
            zact = []     # act counter after ss tanh of chunk i
            pez = []      # pe counter after z matmuls of chunk i
            dyr = []      # dve counter after yr op of chunk i
            dout_i = []   # dout counters per chunk
            relu_d = []

            for i in range(NCHUNK):
                rs = slice(i * 128, (i + 1) * 128)
                # z matmuls: hi (K=128 stacked) then lo-correction (K=64)
                zw = []
                if i < 2:
                    zw = [("act", a_yst), ("dve", d_Llo), ("dmx", cnt["dmx"])]
                else:
                    zw = [("act", zact[i - 2])]
                for c in range(4):
                    pe(zw if c == 0 else None,
                       lambda i=i, c=c: mm(zps[i % 2][:, c * 512:(c + 1) * 512],
                                           Lhh[:, i * 128:(i + 1) * 128],
                                           Rhl[:, c * 512:(c + 1) * 512],
                                           start=True, stop=False))
                for c in range(4):
                    g_z = pe(None,
                             lambda i=i, c=c: mm(zps[i % 2][:, c * 512:(c + 1) * 512],
                                                 Llo[:, i * 128:(i + 1) * 128],
                                                 Rhl[0:2 * D, c * 512:(c + 1) * 512],
                                                 start=False, stop=True),
                             "pe" if c == 3 else None)
                pez.append(g_z)

                # ACT: y = tanh(AA * z)  (fp16 out)
                yw = [("pe", g_z)]
                if i >= 3:
                    yw.append(("dve", dyr[i - 3]))
                zact.append(act(yw, lambda i=i: act_i(ybufs[i % 3][:], zps[i % 2][:],
                                                      AF.Tanh, scale=AA)))

                # DVE: yr = max(y,0); w = yr*DA + CA; out_ss = w*yr; out_st = relu(y_st)
                dyr.append(dve([("act", zact[i])],
                               lambda i=i: V.tensor_scalar_max(yrb[:], ybufs[i % 3][:], 0.0)))
                dve(None, lambda: V.tensor_scalar(wb[:], yrb[:], DA, CA,
                                                  ALU.mult, ALU.add))
                ow = [(f"dout{i % 3}", dout_i[i - 3])] if i >= 3 else [("act", a_yst)]
                dve(ow, lambda i=i: V.tensor_mul(outbufs[i % 3][:, 0:N], wb[:], yrb[:]))
                relu_d.append(dve(None,
                                  lambda i=i: V.tensor_scalar_max(
                                      outbufs[i % 3][:, N:NT],
                                      y_st[:, i * T:(i + 1) * T], 0.0)))
                dout_i.append(op("sync", [("dve", relu_d[i])],
                                 lambda i=i, rs=rs: nc.sync.dma_start(
                                     out=out[rs, :], in_=outbufs[i % 3][:]),
                                 f"dout{i % 3}"))

            # ---------- temporal chunks ----------
            a_prev = None
            for k, (t0, tn) in enumerate(((0, 128), (128, TS - 128))):
                r = NCHUNK + k
                # ts rank-1 matmuls into zA (free after zact[6] / a_ts0)
                tw = [("act", zact[6])] if k == 0 else [("act", a_prev)]
                for c in range(4):
                    pe(tw if c == 0 else None,
                       lambda k=k, c=c, t0=t0, tn=tn: mm(
                           zA[0:tn, c * 512:(c + 1) * 512],
                           s1t[0:1, t0:t0 + tn], ones[0:1, c * 512:(c + 1) * 512],
                           start=True, stop=False))
                for c in range(4):
                    g_ts = pe(None,
                              lambda k=k, c=c, tn=tn: mm(
                                  zA[0:tn, c * 512:(c + 1) * 512],
                                  ones[0:1, 0:tn], s2tb[0:1, c * 512:(c + 1) * 512],
                                  start=False, stop=True),
                              "pe" if c == 3 else None)
                # tt inner products into zB (f32; free after zact[7] / a_tt0)
                ttw = [("act", zact[7])] if k == 0 else [("act", a_prev)]
                g_tt = pe(ttw, lambda t0=t0, tn=tn: mm(
                    zB[0:tn, 0:T], t_in["tmTr32"][:, t0:t0 + tn], t_in["tmT32"][:],
                    start=True, stop=True), "pe")

                # k=1 overwrites ytb/tttb: wait until r=8's DVE reads are done
                tbw = [("dve", relu_d[NCHUNK])] if k == 1 else []
                a_ts = act([("pe", g_ts)] + tbw,
                           lambda tn=tn: act_i(ytb[0:tn, :], zA[0:tn, :], AF.Tanh))
                a_tt = act([("pe", g_tt)], lambda tn=tn: act_i(tttb[0:tn, 0:T],
                                                               zB[0:tn, 0:T], AF.Tanh))
                a_prev = a_tt

                ow = [(f"dout{r % 3}", dout_i[r - 3]), ("act", a_ts)]
                dve(ow, lambda r=r, tn=tn: V.tensor_scalar_max(
                    outbufs[r % 3][0:tn, 0:N], ytb[0:tn, :], 0.0))
                dve([("act", a_tt)], lambda r=r, tn=tn: V.tensor_scalar_max(
                    tttb[0:tn, 0:T], tttb[0:tn, 0:T], 0.0))
                relu_d.append(dve(None, lambda r=r, tn=tn, k=k: V.tensor_mul(
                    outbufs[r % 3][0:tn, N:NT], tttb[0:tn, 0:T],
                    masks[k][0:tn, :])))
                dout_i.append(op("sync", [("dve", relu_d[r])],
                                 lambda r=r, t0=t0, tn=tn: nc.sync.dma_start(
                                     out=out[NS + t0:NS + t0 + tn, :],
                                     in_=outbufs[r % 3][0:tn, :]),
                                 f"dout{r % 3}"))

        # ---------- emit ----------
        with nc.Block() as block:
            def make_body(engine_name):
                ops = plan[engine_name]

                def body(eng):
                    satisfied = {}
                    for waits, fn, inc in ops:
                        for sem_name, val in waits:
                            if val is not None and satisfied.get(sem_name, -1) < val:
                                eng.wait_ge(SEM[sem_name], val)
                                satisfied[sem_name] = val
                        ins = fn()
                        if inc is None:
                            continue
                        if inc == "din" or inc == "dmx" or inc.startswith("dout"):
                            ins.then_inc(SEM[inc], 16)
                        else:
                            ins.then_inc(SEM[inc], 1)
                return body

            block.sync(make_body("sync"))
            block.tensor(make_body("tensor"))
            block.scalar(make_body("scalar"))
            block.vector(make_body("vector"))

    return nc


def build_in_maps(spatial_nodes, temporal_nodes, W_ss1, W_ss2, w_st, b_st, w_ts, b_ts):
    f, h = np.float32, np.float16
    W12T_R = np.ascontiguousarray(np.concatenate([W_ss2.T, W_ss1.T], axis=1), dtype=f)
    W12T_L = np.ascontiguousarray(np.concatenate([W_ss1.T, -W_ss2.T], axis=1), dtype=f)
    stind = np.zeros((9, NCHUNK * T), dtype=h)
    for k in range(NCHUNK):
        stind[k + 1, k * T:(k + 1) * T] = 1.0
    in_maps = []
    for c in range(N_CORES):
        b, hh = divmod(c, 2)
        tmask = (np.arange(T)[None, :] >= (hh * TS + np.arange(TS))[:, None]).astype(h)
        spT = np.ascontiguousarray(spatial_nodes[b].T, dtype=f)
        tmT = np.ascontiguousarray(temporal_nodes[b].T, dtype=f)
        spTr = np.ascontiguousarray(spT[:, hh * NS:(hh + 1) * NS])
        tmTr = np.ascontiguousarray(tmT[:, hh * TS:(hh + 1) * TS])
        in_maps.append({
            "spT32": spT, "spT16": spT.astype(h),
            "spTr32": spTr, "spTr16": spTr.astype(h),
            "tmT32": tmT, "tmT16": tmT.astype(h),
            "tmTr32": tmTr, "tmTr16": tmTr.astype(h),
            "W12T_R": W12T_R, "W12T_L": W12T_L,
            "wst_a16": np.ascontiguousarray(w_st[:D, None], dtype=h),
            "wst_b16": np.ascontiguousarray(w_st[D:, None], dtype=h),
            "wts_a16": np.ascontiguousarray(w_ts[:D, None], dtype=h),
            "wts_b16": np.ascontiguousarray(w_ts[D:, None], dtype=h),
            "bst": np.asarray(b_st, dtype=f).reshape(1, 1),
            "bts": np.asarray(b_ts, dtype=f).reshape(1, 1),
            "ttmask16": tmask,
            "stind": stind,
        })
    return in_maps


def assemble(results):
    out = np.empty((B, NT, NT), np.float32)
    for c in range(N_CORES):
        b, h = divmod(c, 2)
        r = results[c]["out"].astype(np.float32)
        out[b, h * NS:(h + 1) * NS, :] = r[0:NS]
        out[b, N + h * TS: N + (h + 1) * TS, :] = r[NS:ROWS]
    return out


_NC = None


def kernel(**inputs):
    global _NC
    if _NC is None:
        _NC = build_program()
    in_maps = build_in_maps(**inputs)
    res = run_bass_kernel_spmd(_NC, in_maps, list(range(N_CORES)))
    return assemble(res.results)


# revision 18
# speedup vs baseline: 1.9216x; 1.7007x over previous
"""Trainium2 Bass kernel for the MLPSim adjacency-constructor problem.

Full shapes: spatial [4, 2048, 32], temporal [4, 288, 32], output
adj [4, 2336, 2336] f32 where adj = tanh(relu(blocks)):
  ss = tanh(m - m^T), m = nv1 @ nv2^T, nv_i = tanh(3*x@W_i^T)
  st = s1[n] + s2[t] + b_st ;  ts = s1t[t] + s2t[n] + b_ts
  tt = triu(temporal @ temporal^T)

Sharding: 8 cores = (batch b = c//2) x (row-half h = c%2); each core emits
1024 spatial + 144 temporal rows ([1168, 2336]) of one batch, stored fp16
on device and upcast to f32 on the host during unshard.

Device algebra (ACT is the wall: 1 elem/lane/cycle, dtype-independent):
  ss out = tanh(tanh(relu(z))), z = m - m^T. Minimax fit
    tanh(tanh(relu(z))) ~= max((C + D*y)*y, 0),  y = tanh(A*z)
  with A=1.15118303 C=0.90136458 D=-0.141975 (fit err 2.2e-3; (C+D*y)*y is
  negative whenever y<0, so the final relu also zeroes the z<0 half) folds
  the two full-size ACT tanh passes into ONE; the quadratic runs on DVE in
  3 fused 16-bit ops. z is a plain fp16 matmul (nv exact, one fp16
  rounding -> ~7e-3 total err vs the 2e-2 gate). nv pre-acts u = x@W are
  an exact bf16 hi/lo K-stacked matmul ([Wh;Wh;Wl].T @ [xh;xl;xh], err
  ~1e-5), avoiding slow f32 matmuls. st is one K=9 fp16 matmul (chunk-
  indicator rows + s2 row) + one ACT pass covering all 8 chunks; ts is
  K=1 rank-1 fp16 matmuls; tt stays f32 (only 288 cols).
"""

import numpy as np
from contextlib import ExitStack

import concourse.bass as bass
from concourse import mybir
from concourse.bass_utils import run_bass_kernel_spmd

AF = mybir.ActivationFunctionType
ALU = mybir.AluOpType
F32 = mybir.dt.float32
F16 = mybir.dt.float16
BF16 = mybir.dt.bfloat16

B, N, T, D = 4, 2048, 288, 32
NS = N // 2
TS = T // 2
NT = N + T
ROWS = NS + TS
N_CORES = 8
NCHUNK = NS // 128

AA = 1.15118303
CA = 0.90136458
DA = -0.141975

GROUP_A = ("sp96", "W96_R", "sp96r", "W96_L")


def build_program():
    nc = bass.Bass()
    inp = {}

    for name, shape, dt in (
        ("sp96", (3 * D, N), BF16), ("W96_R", (3 * D, 2 * D), BF16),
        ("sp96r", (3 * D, NS), BF16), ("W96_L", (3 * D, 2 * D), BF16),
        ("spT16", (D, N), F16), ("spTr16", (D, NS), F16),
        ("tmT16", (D, T), F16), ("tmTr16", (D, TS), F16),
        ("tmT32", (D, T), F32), ("tmTr32", (D, TS), F32),
        ("wst_a16", (D, 1), F16), ("wst_b16", (D, 1), F16),
        ("wts_a16", (D, 1), F16), ("wts_b16", (D, 1), F16),
        ("bst", (1, 1), F32), ("bts", (1, 1), F32),
        ("ttmask16", (TS, T), F16), ("stind", (9, NCHUNK * T), F16),
    ):
        inp[name] = nc.declare_dram_parameter(name, list(shape), dt, isOutput=False)
    out = nc.declare_dram_parameter("out", [ROWS, NT], F16, isOutput=True)

    ctx = ExitStack()
    _uid = [0]

    def sbuf(shape, dt=F16):
        _uid[0] += 1
        return ctx.enter_context(nc.sbuf_tensor(f"sb{_uid[0]}", list(shape), dt))

    with ctx:
        t_in = {k: sbuf(v.shape, v.dtype) for k, v in inp.items() if k != "ttmask16"}
        masks = [sbuf([128, T]), sbuf([TS - 128, T])]
        Rf16 = sbuf([2 * D, N])       # [nv2; nv1] fp16, all N nodes
        Lf16 = sbuf([2 * D, NS])      # [nv1; -nv2] fp16, rows-half
        s2row = sbuf([1, T])
        s1t16 = sbuf([1, TS])
        s2tb = sbuf([1, N])
        ones = sbuf([1, N])
        stL = sbuf([9, 128])
        s1row = sbuf([1, NS])
        y_st = sbuf([128, NCHUNK * T])
        ybufs = [sbuf([128, N]) for _ in range(3)]
        yrb = sbuf([128, N])
        wb = sbuf([128, N])
        ytb = sbuf([128, N])
        tttb = sbuf([128, T])
        scr = sbuf([1, 8], F32)
        outbufs = [sbuf([128, NT]) for _ in range(3)]

        dina = ctx.enter_context(nc.semaphore("dina"))
        dinb = ctx.enter_context(nc.semaphore("dinb"))
        dmx_s = ctx.enter_context(nc.semaphore("dmx"))
        pe_s = ctx.enter_context(nc.semaphore("pe_s"))
        act_s = ctx.enter_context(nc.semaphore("act_s"))
        dve_s = ctx.enter_context(nc.semaphore("dve_s"))
        douts = [ctx.enter_context(nc.semaphore(f"dout{k}")) for k in range(3)]
        SEM = {"pe": pe_s, "act": act_s, "dve": dve_s, "dina": dina, "dinb": dinb,
               "dmx": dmx_s, "dout0": douts[0], "dout1": douts[1], "dout2": douts[2]}

        plan = {"sync": [], "tensor": [], "scalar": [], "vector": []}
        cnt = {k: 0 for k in SEM}

        def op(engine, waits, fn, inc=None):
            plan[engine].append((waits or [], fn, inc))
            if inc:
                cnt[inc] += 16 if inc.startswith("d") and inc != "dve" else 1
                return cnt[inc]
            return None

        def pe(waits, fn, inc=None):
            return op("tensor", waits, fn, inc)

        def act(waits, fn):
            return op("scalar", waits, fn, "act")

        def dve(waits, fn):
            return op("vector", waits, fn, "dve")

        mm = nc.tensor.matmul
        act_i = nc.scalar.activation
        V = nc.vector

        # ---------- input loads (group A gates the first matmuls) ----------
        for name in GROUP_A:
            op("sync", None, lambda t=t_in[name], s=inp[name]:
               nc.sync.dma_start(out=t[:], in_=s[:]), "dina")
        dina_all = cnt["dina"]
        for name, tt_ in t_in.items():
            if name in GROUP_A:
                continue
            op("sync", None, lambda t=tt_, s=inp[name]:
               nc.sync.dma_start(out=t[:], in_=s[:]), "dinb")
        op("sync", None, lambda: nc.sync.dma_start(out=masks[0][:],
                                                   in_=inp["ttmask16"][0:128, :]), "dinb")
        op("sync", None, lambda: nc.sync.dma_start(out=masks[1][:],
                                                   in_=inp["ttmask16"][128:TS, :]), "dinb")
        dinb_all = cnt["dinb"]

        # ACT: load the tanh table right away (no data deps)
        act(None, lambda: act_i(scr[:], scr[:], AF.Tanh))
        # DVE: constant tiles (no deps)
        dve(None, lambda: V.memset(ones[:], 1.0))
        dve(None, lambda: V.memset(stL[0:1, :], 1.0))

        # ================= PREP A: pu [64,2048] + sv [1,2048] ===============
        with nc.psum_tensor("pu", [2 * D, N], F32) as pu, \
             nc.psum_tensor("sv", [1, N], F32) as sv:
            # nv pre-acts, exact bf16 hi/lo stack
            for c in range(4):
                g_pu = pe([("dina", dina_all)] if c == 0 else None,
                          lambda c=c: mm(pu[:, c * 512:(c + 1) * 512], t_in["W96_R"][:],
                                         t_in["sp96"][:, c * 512:(c + 1) * 512],
                                         start=True, stop=True), "pe" if c == 3 else None)
            # svec round 1 (fp16): s1 | s2 | s1t packed into sv columns
            pe([("dinb", dinb_all)], lambda: mm(sv[0:1, 0:512], t_in["wst_a16"][:],
                                                t_in["spTr16"][:, 0:512],
                                                start=True, stop=True))
            pe(None, lambda: mm(sv[0:1, 512:1024], t_in["wst_a16"][:],
                                t_in["spTr16"][:, 512:1024], start=True, stop=True))
            pe(None, lambda: mm(sv[0:1, 1024:1024 + T], t_in["wst_b16"][:],
                                t_in["tmT16"][:], start=True, stop=True))
            g_sv1 = pe(None, lambda: mm(sv[0:1, 1312:1312 + TS], t_in["wts_a16"][:],
                                        t_in["tmTr16"][:], start=True, stop=True), "pe")

            a_Rf = act([("pe", g_pu)], lambda: act_i(Rf16[:], pu[:], AF.Tanh, scale=3.0))

            d_s1 = dve([("pe", g_sv1)], lambda: V.tensor_copy(s1row[:], sv[0:1, 0:NS]))
            d_s2 = dve(None, lambda: V.tensor_scalar_add(s2row[:], sv[0:1, 1024:1024 + T],
                                                         t_in["bst"][0:1, 0:1]))
            d_s1t = dve(None, lambda: V.tensor_copy(s1t16[:], sv[0:1, 1312:1312 + TS]))

            # Lf into pu banks 0-1 (freed by a_Rf)
            for c in range(2):
                g_pv = pe([("act", a_Rf)] if c == 0 else None,
                          lambda c=c: mm(pu[:, c * 512:(c + 1) * 512], t_in["W96_L"][:],
                                         t_in["sp96r"][:, c * 512:(c + 1) * 512],
                                         start=True, stop=True), "pe" if c == 1 else None)
            a_Lf = act([("pe", g_pv)], lambda: act_i(Lf16[:], pu[:, 0:NS], AF.Tanh,
                                                     scale=3.0))

            # svec round 2: s2t over all N (overwrites sv; waits round-1 reads)
            for c in range(4):
                g_sv2 = pe([("dve", d_s1t)] if c == 0 else None,
                           lambda c=c: mm(sv[0:1, c * 512:(c + 1) * 512],
                                          t_in["wts_b16"][:],
                                          t_in["spT16"][:, c * 512:(c + 1) * 512],
                                          start=True, stop=True), "pe" if c == 3 else None)
            d_s2t = dve([("pe", g_sv2)], lambda: V.tensor_scalar_add(
                s2tb[:], sv[0:1, :], t_in["bts"][0:1, 0:1]))

        # aux DMAs: stL rows 1-8 (s1 reshaped) and stR row 0 (s2 tiled)
        stR = t_in["stind"]
        op("sync", [("dve", d_s1)],
           lambda: nc.sync.dma_start(out=stL[1:9, :], in_=s1row[:]), "dmx")
        for k in range(NCHUNK):
            op("sync", [("dve", d_s2)],
               lambda k=k: nc.sync.dma_start(out=stR[0:1, k * T:(k + 1) * T],
                                             in_=s2row[:]), "dmx")
        dmx_all = cnt["dmx"]

        # ================= PREP B: stp [128, 2304] ==========================
        with nc.psum_tensor("stp", [128, NCHUNK * T], F32) as stp:
            npc = NCHUNK * T
            stw = [("act", a_Lf), ("dve", d_s2t), ("dmx", dmx_all)]
            for c in range(5):
                c0, c1 = c * 512, min((c + 1) * 512, npc)
                g_stp = pe(stw if c == 0 else None,
                           lambda c0=c0, c1=c1: mm(stp[:, c0:c1], stL[:], stR[:, c0:c1],
                                                   start=True, stop=True),
                           "pe" if c == 4 else None)
            a_yst = act([("pe", g_stp)], lambda: act_i(y_st[:], stp[:], AF.Tanh))

        # ================= MAIN: zA + zB [128, 2048] ========================
        with nc.psum_tensor("zA", [128, N], F32) as zA, \
             nc.psum_tensor("zB", [128, N], F32) as zB:
            zps = [zA, zB]
            zact, pez, dyr, dout_i, relu_d = [], [], [], [], []

            for i in range(NCHUNK):
                rs = slice(i * 128, (i + 1) * 128)
                zw = [("act", a_yst)] if i < 2 else [("act", zact[i - 2])]
                for c in range(4):
                    g_z = pe(zw if c == 0 else None,
                             lambda i=i, c=c: mm(zps[i % 2][:, c * 512:(c + 1) * 512],
                                                 Lf16[:, i * 128:(i + 1) * 128],
                                                 Rf16[:, c * 512:(c + 1) * 512],
                                                 start=True, stop=True),
                             "pe" if c == 3 else None)
                pez.append(g_z)

                yw = [("pe", g_z)] + ([("dve", dyr[i - 3])] if i >= 3 else [])
                zact.append(act(yw, lambda i=i: act_i(ybufs[i % 3][:], zps[i % 2][:],
                                                      AF.Tanh, scale=AA)))

                dyr.append(dve([("act", zact[i])],
                               lambda i=i: V.tensor_scalar_max(yrb[:], ybufs[i % 3][:],
                                                               0.0)))
                dve(None, lambda: V.tensor_scalar(wb[:], yrb[:], DA, CA,
                                                  ALU.mult, ALU.add))
                ow = [(f"dout{i % 3}", dout_i[i - 3])] if i >= 3 else [("act", a_yst)]
                dve(ow, lambda i=i: V.tensor_mul(outbufs[i % 3][:, 0:N], wb[:], yrb[:]))
                relu_d.append(dve(None, lambda i=i: V.tensor_scalar_max(
                    outbufs[i % 3][:, N:NT], y_st[:, i * T:(i + 1) * T], 0.0)))
                dout_i.append(op("sync", [("dve", relu_d[i])],
                                 lambda i=i, rs=rs: nc.sync.dma_start(
                                     out=out[rs, :], in_=outbufs[i % 3][:]),
                                 f"dout{i % 3}"))

            # ---------- temporal rows ----------
            a_ts_p, a_tt_p = None, None
            for k, (t0, tn) in enumerate(((0, 128), (128, TS - 128))):
                r = NCHUNK + k
                tw = [("act", zact[6])] if k == 0 else [("act", a_ts_p)]
                for c in range(4):
                    pe(tw if c == 0 else None,
                       lambda c=c, t0=t0, tn=tn: mm(zA[0:tn, c * 512:(c + 1) * 512],
                                                    s1t16[0:1, t0:t0 + tn],
                                                    ones[0:1, c * 512:(c + 1) * 512],
                                                    start=True, stop=False))
                    g_ts = pe(None,
                              lambda c=c, tn=tn: mm(zA[0:tn, c * 512:(c + 1) * 512],
                                                    ones[0:1, 0:tn],
                                                    s2tb[0:1, c * 512:(c + 1) * 512],
                                                    start=False, stop=True),
                              "pe" if c == 3 else None)
                ttw = [("act", zact[7])] if k == 0 else [("act", a_tt_p)]
                g_tt = pe(ttw, lambda t0=t0, tn=tn: mm(
                    zB[0:tn, 0:T], t_in["tmTr32"][:, t0:t0 + tn], t_in["tmT32"][:],
                    start=True, stop=True), "pe")

                tbw = [("dve", relu_d[NCHUNK])] if k == 1 else []
                a_ts_p = act([("pe", g_ts)] + tbw,
                             lambda tn=tn: act_i(ytb[0:tn, :], zA[0:tn, :], AF.Tanh))
                a_tt_p = act([("pe", g_tt)], lambda tn=tn: act_i(tttb[0:tn, 0:T],
                                                                 zB[0:tn, 0:T], AF.Tanh))

                ow = [(f"dout{r % 3}", dout_i[r - 3]), ("act", a_ts_p)]
                dve(ow, lambda r=r, tn=tn: V.tensor_scalar_max(
                    outbufs[r % 3][0:tn, 0:N], ytb[0:tn, :], 0.0))
                dve([("act", a_tt_p)], lambda tn=tn: V.tensor_scalar_max(
                    tttb[0:tn, 0:T], tttb[0:tn, 0:T], 0.0))
                relu_d.append(dve(None, lambda r=r, tn=tn, k=k: V.tensor_mul(
                    outbufs[r % 3][0:tn, N:NT], tttb[0:tn, 0:T], masks[k][0:tn, :])))
                dout_i.append(op("sync", [("dve", relu_d[r])],
                                 lambda r=r, t0=t0, tn=tn: nc.sync.dma_start(
                                     out=out[NS + t0:NS + t0 + tn, :],
                                     in_=outbufs[r % 3][0:tn, :]),
                                 f"dout{r % 3}"))

        # ---------- emit ----------
        with nc.Block() as block:
            def make_body(engine_name):
                ops = plan[engine_name]

                def body(eng):
                    satisfied = {}
                    for waits, fn, inc in ops:
                        for sem_name, val in waits:
                            if val is not None and satisfied.get(sem_name, -1) < val:
                                eng.wait_ge(SEM[sem_name], val)
                                satisfied[sem_name] = val
                        ins = fn()
                        if inc is None:
                            continue
                        if inc in ("dina", "dinb", "dmx") or inc.startswith("dout"):
                            ins.then_inc(SEM[inc], 16)
                        else:
                            ins.then_inc(SEM[inc], 1)
                return body

            block.sync(make_body("sync"))
            block.tensor(make_body("tensor"))
            block.scalar(make_body("scalar"))
            block.vector(make_body("vector"))

    return nc


def _bf16(x):
    # round-to-nearest bf16 via float32 bit manipulation, returned as float32
    u = x.astype(np.float32).view(np.uint32)
    r = ((u >> 16) + ((u >> 15) & 1)).astype(np.uint32) << 16
    return r.view(np.float32)


def build_in_maps(spatial_nodes, temporal_nodes, W_ss1, W_ss2, w_st, b_st, w_ts, b_ts):
    import ml_dtypes
    f, h = np.float32, np.float16
    bf = ml_dtypes.bfloat16

    def stack96(a32):
        hi = _bf16(a32)
        lo = _bf16(a32 - hi)
        return np.ascontiguousarray(np.concatenate([hi, lo, hi], axis=0)).astype(bf)

    def stackW(w32):
        hi = _bf16(w32)
        lo = _bf16(w32 - hi)
        return np.ascontiguousarray(np.concatenate([hi, hi, lo], axis=0)).astype(bf)

    W_R = np.concatenate([W_ss2.T, W_ss1.T], axis=1).astype(f)
    W_L = np.concatenate([W_ss1.T, -W_ss2.T], axis=1).astype(f)
    W96_R = stackW(W_R)
    W96_L = stackW(W_L)
    stind = np.zeros((9, NCHUNK * T), dtype=h)
    for k in range(NCHUNK):
        stind[k + 1, k * T:(k + 1) * T] = 1.0
    in_maps = []
    for c in range(N_CORES):
        b, hh = divmod(c, 2)
        tmask = (np.arange(T)[None, :] >= (hh * TS + np.arange(TS))[:, None]).astype(h)
        spT = np.ascontiguousarray(spatial_nodes[b].T, dtype=f)
        tmT = np.ascontiguousarray(temporal_nodes[b].T, dtype=f)
        spTr = np.ascontiguousarray(spT[:, hh * NS:(hh + 1) * NS])
        tmTr = np.ascontiguousarray(tmT[:, hh * TS:(hh + 1) * TS])
        sp96 = stack96(spT)
        in_maps.append({
            "sp96": sp96, "W96_R": W96_R,
            "sp96r": np.ascontiguousarray(sp96[:, hh * NS:(hh + 1) * NS]),
            "W96_L": W96_L,
            "spT16": spT.astype(h), "spTr16": spTr.astype(h),
            "tmT16": tmT.astype(h), "tmTr16": tmTr.astype(h),
            "tmT32": tmT, "tmTr32": tmTr,
            "wst_a16": np.ascontiguousarray(w_st[:D, None], dtype=h),
            "wst_b16": np.ascontiguousarray(w_st[D:, None], dtype=h),
            "wts_a16": np.ascontiguousarray(w_ts[:D, None], dtype=h),
            "wts_b16": np.ascontiguousarray(w_ts[D:, None], dtype=h),
            "bst": np.asarray(b_st, dtype=f).reshape(1, 1),
            "bts": np.asarray(b_ts, dtype=f).reshape(1, 1),
            "ttmask16": tmask,
            "stind": stind,
        })
    return in_maps


def assemble(results):
    out = np.empty((B, NT, NT), np.float32)
    for c in range(N_CORES):
        b, h = divmod(c, 2)
        r = results[c]["out"].astype(np.float32)
        out[b, h * NS:(h + 1) * NS, :] = r[0:NS]
        out[b, N + h * TS: N + (h + 1) * TS, :] = r[NS:ROWS]
    return out


_NC = None


def kernel(**inputs):
    global _NC
    if _NC is None:
        _NC = build_program()
    in_maps = build_in_maps(**inputs)
    res = run_bass_kernel_spmd(_NC, in_maps, list(range(N_CORES)))
    return assemble(res.results)


# revision 24
# speedup vs baseline: 2.0027x; 1.0422x over previous
"""Trainium2 Bass kernel for the MLPSim adjacency-constructor problem.

Full shapes: spatial [4, 2048, 32], temporal [4, 288, 32], output
adj [4, 2336, 2336] f32 where adj = tanh(relu(blocks)):
  ss = tanh(m - m^T), m = nv1 @ nv2^T, nv_i = tanh(3*x@W_i^T)
  st = s1[n] + s2[t] + b_st ;  ts = s1t[t] + s2t[n] + b_ts
  tt = triu(temporal @ temporal^T)

Sharding: 8 cores = (batch b = c//2) x (row-half h = c%2); each core emits
1024 spatial + 144 temporal rows ([1168, 2336]) of one batch, stored fp16
on device and upcast to f32 on the host during unshard.

Device algebra (ACT is the wall: 1 elem/lane/cycle, dtype-independent):
  ss out = tanh(tanh(relu(z))), z = m - m^T. Minimax fit
    tanh(tanh(relu(z))) ~= max((C + D*y)*y, 0),  y = tanh(A*z)
  with A=1.15118303 C=0.90136458 D=-0.141975 (fit err 2.2e-3; (C+D*y)*y is
  negative whenever y<0, so the final relu also zeroes the z<0 half) folds
  the two full-size ACT tanh passes into ONE; the quadratic runs on DVE in
  3 fused 16-bit ops. z is a plain fp16 matmul (nv exact, one fp16
  rounding -> ~7e-3 total err vs the 2e-2 gate). nv pre-acts u = x@W are
  an exact bf16 hi/lo K-stacked matmul ([Wh;Wh;Wl].T @ [xh;xl;xh], err
  ~1e-5), avoiding slow f32 matmuls. st is a K=9 fp16 matmul (chunk-
  indicator rows) plus rank-1 s2 accumulate-matmuls + ONE ACT pass for
  all 8 chunks; ts is a K=2 rank-1 fp16 matmul; tt stays f32 (288 cols).
  Temporal passes interleave into the tail of the spatial ACT stream.
"""

import numpy as np
from contextlib import ExitStack

import concourse.bass as bass
from concourse import mybir
from concourse.bass_utils import run_bass_kernel_spmd

AF = mybir.ActivationFunctionType
ALU = mybir.AluOpType
F32 = mybir.dt.float32
F16 = mybir.dt.float16
BF16 = mybir.dt.bfloat16

B, N, T, D = 4, 2048, 288, 32
NS = N // 2
TS = T // 2
NT = N + T
ROWS = NS + TS
N_CORES = 8
NCHUNK = NS // 128

AA = 1.15118303
CA = 0.90136458
DA = -0.141975

G_A = ("sp96", "W96_R")
G_L = ("sp96r", "W96_L")
G_C = ("tmT32", "tmTr32", "stind")


def build_program():
    nc = bass.Bass()
    inp = {}

    for name, shape, dt in (
        ("sp96", (3 * D, N), BF16), ("W96_R", (3 * D, 2 * D), BF16),
        ("sp96r", (3 * D, NS), BF16), ("W96_L", (3 * D, 2 * D), BF16),
        ("spT16", (D, N), F16), ("spTr16", (D, NS), F16),
        ("tmT16", (D, T), F16), ("tmTr16", (D, TS), F16),
        ("tmT32", (D, T), F32), ("tmTr32", (D, TS), F32),
        ("wst_a16", (D, 1), F16), ("wst_b16", (D, 1), F16),
        ("wts_a16", (D, 1), F16), ("wts_b16", (D, 1), F16),
        ("bst", (1, 1), F32), ("bts", (1, 1), F32),
        ("ttmask16", (TS, T), F16), ("stind", (9, NCHUNK * T), F16),
    ):
        inp[name] = nc.declare_dram_parameter(name, list(shape), dt, isOutput=False)
    out = nc.declare_dram_parameter("out", [ROWS, NT], F16, isOutput=True)

    ctx = ExitStack()
    _uid = [0]

    def sbuf(shape, dt=F16):
        _uid[0] += 1
        return ctx.enter_context(nc.sbuf_tensor(f"sb{_uid[0]}", list(shape), dt))

    with ctx:
        t_in = {k: sbuf(v.shape, v.dtype) for k, v in inp.items() if k != "ttmask16"}
        masks = [sbuf([128, T]), sbuf([TS - 128, T])]
        Rf16 = sbuf([2 * D, N])
        Lf16 = sbuf([2 * D, NS])
        s2row = sbuf([1, T])
        s2tb = sbuf([1, N])
        ones = sbuf([1, N])
        stL = sbuf([9, 128])
        s1row = sbuf([1, NS])
        tsL = sbuf([2, TS])           # [s1t ; ones]
        tsR = sbuf([2, N])            # [ones ; s2t+b]
        y_st = sbuf([128, NCHUNK * T])
        ybufs = [sbuf([128, N]) for _ in range(3)]
        yrb = sbuf([128, N])
        wb = sbuf([128, N])
        ytb = sbuf([128, N])
        tttb = sbuf([128, T])
        ttres = [sbuf([128, T]), sbuf([TS - 128, T])]
        scr = sbuf([1, 8], F32)
        outbufs = [sbuf([128, NT]) for _ in range(3)]

        sems = {}
        for sname in ("dina", "dinl", "dinb", "dinc", "dmx", "pe_s", "act_s",
                      "dve_s", "dout0", "dout1", "dout2"):
            sems[sname] = ctx.enter_context(nc.semaphore(sname))
        SEM = {"pe": sems["pe_s"], "act": sems["act_s"], "dve": sems["dve_s"],
               "dina": sems["dina"], "dinl": sems["dinl"], "dinb": sems["dinb"],
               "dinc": sems["dinc"], "dmx": sems["dmx"],
               "dout0": sems["dout0"], "dout1": sems["dout1"], "dout2": sems["dout2"]}

        plan = {"sync": [], "tensor": [], "scalar": [], "vector": []}
        cnt = {k: 0 for k in SEM}

        def op(engine, waits, fn, inc=None):
            plan[engine].append((waits or [], fn, inc))
            if inc:
                cnt[inc] += 1 if inc in ("pe", "act", "dve") else 16
                return cnt[inc]
            return None

        def pe(waits, fn, inc=None):
            return op("tensor", waits, fn, inc)

        def act(waits, fn):
            return op("scalar", waits, fn, "act")

        def dve(waits, fn):
            return op("vector", waits, fn, "dve")

        mm = nc.tensor.matmul
        act_i = nc.scalar.activation
        V = nc.vector

        # ---------- input loads, finest-grained gating first ----------
        def load(name, grp):
            op("sync", None, lambda t=t_in[name], s=inp[name]:
               nc.sync.dma_start(out=t[:], in_=s[:]), grp)

        for name in G_A:
            load(name, "dina")
        dina_all = cnt["dina"]
        for name in G_L:
            load(name, "dinl")
        dinl_all = cnt["dinl"]
        for name in ("spTr16", "tmT16", "tmTr16", "wst_a16", "wst_b16",
                     "wts_a16", "wts_b16", "bst", "bts", "spT16"):
            load(name, "dinb")
        dinb_all = cnt["dinb"]
        for name in G_C:
            load(name, "dinc")
        op("sync", None, lambda: nc.sync.dma_start(out=masks[0][:],
                                                   in_=inp["ttmask16"][0:128, :]), "dinc")
        op("sync", None, lambda: nc.sync.dma_start(out=masks[1][:],
                                                   in_=inp["ttmask16"][128:TS, :]), "dinc")
        dinc_all = cnt["dinc"]

        # ACT: load the tanh table right away; DVE: constant tiles
        act(None, lambda: act_i(scr[:], scr[:], AF.Tanh))
        d_ones = dve(None, lambda: V.memset(ones[:], 1.0))
        dve(None, lambda: V.memset(stL[0:1, :], 1.0))
        dve(None, lambda: V.memset(tsR[0:1, :], 1.0))

        # ================= PREP A: pu [64,2048] + sv [1,2048] ===============
        with nc.psum_tensor("pu", [2 * D, N], F32) as pu, \
             nc.psum_tensor("sv", [1, N], F32) as sv:
            for c in range(4):
                g_pu = pe([("dina", dina_all)] if c == 0 else None,
                          lambda c=c: mm(pu[:, c * 512:(c + 1) * 512], t_in["W96_R"][:],
                                         t_in["sp96"][:, c * 512:(c + 1) * 512],
                                         start=True, stop=True), "pe" if c == 3 else None)
            pe([("dinb", dinb_all)], lambda: mm(sv[0:1, 0:512], t_in["wst_a16"][:],
                                                t_in["spTr16"][:, 0:512],
                                                start=True, stop=True))
            pe(None, lambda: mm(sv[0:1, 512:1024], t_in["wst_a16"][:],
                                t_in["spTr16"][:, 512:1024], start=True, stop=True))
            pe(None, lambda: mm(sv[0:1, 1024:1024 + T], t_in["wst_b16"][:],
                                t_in["tmT16"][:], start=True, stop=True))
            g_sv1 = pe(None, lambda: mm(sv[0:1, 1312:1312 + TS], t_in["wts_a16"][:],
                                        t_in["tmTr16"][:], start=True, stop=True), "pe")

            a_Rf = act([("pe", g_pu)], lambda: act_i(Rf16[:], pu[:], AF.Tanh, scale=3.0))

            d_s1 = dve([("pe", g_sv1)], lambda: V.tensor_copy(s1row[:], sv[0:1, 0:NS]))
            d_s2 = dve(None, lambda: V.tensor_scalar_add(s2row[:], sv[0:1, 1024:1024 + T],
                                                         t_in["bst"][0:1, 0:1]))
            d_s1t = dve(None, lambda: V.tensor_copy(tsL[0:1, :], sv[0:1, 1312:1312 + TS]))

            for c in range(2):
                g_pv = pe([("act", a_Rf), ("dinl", dinl_all)] if c == 0 else None,
                          lambda c=c: mm(pu[:, c * 512:(c + 1) * 512], t_in["W96_L"][:],
                                         t_in["sp96r"][:, c * 512:(c + 1) * 512],
                                         start=True, stop=True), "pe" if c == 1 else None)
            a_Lf = act([("pe", g_pv)], lambda: act_i(Lf16[:], pu[:, 0:NS], AF.Tanh,
                                                     scale=3.0))

            for c in range(4):
                g_sv2 = pe([("dve", d_s1t)] if c == 0 else None,
                           lambda c=c: mm(sv[0:1, c * 512:(c + 1) * 512],
                                          t_in["wts_b16"][:],
                                          t_in["spT16"][:, c * 512:(c + 1) * 512],
                                          start=True, stop=True), "pe" if c == 3 else None)
            d_s2t = dve([("pe", g_sv2)], lambda: V.tensor_scalar_add(
                s2tb[:], sv[0:1, :], t_in["bts"][0:1, 0:1]))

        # aux DMAs (all overlap later compute)
        x_s1 = op("sync", [("dve", d_s1)],
                  lambda: nc.sync.dma_start(out=stL[1:9, :], in_=s1row[:]), "dmx")
        op("sync", [("dve", d_ones)],
           lambda: nc.sync.dma_start(out=tsL[1:2, :], in_=ones[0:1, 0:TS]), "dmx")
        x_tsr = op("sync", [("dve", d_s2t)],
                   lambda: nc.sync.dma_start(out=tsR[1:2, :], in_=s2tb[:]), "dmx")

        # ====== PREP B: stp [128, 2304] + ttp [128, 288] (tt done early) ====
        with nc.psum_tensor("stp", [128, NCHUNK * T], F32) as stp, \
             nc.psum_tensor("ttp", [128, T], F32) as ttp:
            npc = NCHUNK * T
            stw = [("act", a_Lf), ("dve", d_s2t), ("dmx", x_s1), ("dinc", dinc_all)]
            for c in range(5):
                c0, c1 = c * 512, min((c + 1) * 512, npc)
                pe(stw if c == 0 else None,
                   lambda c0=c0, c1=c1: mm(stp[:, c0:c1], stL[:],
                                           t_in["stind"][:, c0:c1],
                                           start=True, stop=False))
            for k in range(NCHUNK):
                g_stp = pe(None,
                           lambda k=k: mm(stp[:, k * T:(k + 1) * T], ones[0:1, 0:128],
                                          s2row[:], start=False, stop=True),
                           "pe" if k == NCHUNK - 1 else None)
            a_yst = act([("pe", g_stp)], lambda: act_i(y_st[:], stp[:], AF.Tanh))

            # tt block, fully staged into ttres during prep
            g_tt0 = pe(None, lambda: mm(ttp[0:128, :], t_in["tmTr32"][:, 0:128],
                                        t_in["tmT32"][:], start=True, stop=True), "pe")
            a_tt0 = act([("pe", g_tt0)], lambda: act_i(tttb[0:128, :], ttp[0:128, :],
                                                       AF.Tanh))
            tn = TS - 128
            g_tt1 = pe([("act", a_tt0)],
                       lambda tn=tn: mm(ttp[0:tn, :], t_in["tmTr32"][:, 128:TS],
                                        t_in["tmT32"][:], start=True, stop=True), "pe")
            # ttres = max(tanh,0) * triu-mask, one fused DVE op each
            d_tt0 = dve([("act", a_tt0), ("dinc", dinc_all)],
                        lambda: V.scalar_tensor_tensor(ttres[0][:], tttb[0:128, :], 0.0,
                                                       masks[0][:], ALU.max, ALU.mult))
            a_tt1 = act([("pe", g_tt1), ("dve", d_tt0)],
                        lambda tn=tn: act_i(tttb[0:tn, :], ttp[0:tn, :], AF.Tanh))
            dve([("act", a_tt1)],
                lambda tn=tn: V.scalar_tensor_tensor(ttres[1][:], tttb[0:tn, :], 0.0,
                                                     masks[1][:], ALU.max, ALU.mult))

        # ================= MAIN: zA + zB [128, 2048] ========================
        with nc.psum_tensor("zA", [128, N], F32) as zA, \
             nc.psum_tensor("zB", [128, N], F32) as zB:
            zps = [zA, zB]
            zact, dyr, dout_i, relu_d = [], [], [], []

            for i in range(NCHUNK):
                rs = slice(i * 128, (i + 1) * 128)
                zw = [("act", a_yst)] if i < 2 else [("act", zact[i - 2])]
                for c in range(4):
                    g_z = pe(zw if c == 0 else None,
                             lambda i=i, c=c: mm(zps[i % 2][:, c * 512:(c + 1) * 512],
                                                 Lf16[:, i * 128:(i + 1) * 128],
                                                 Rf16[:, c * 512:(c + 1) * 512],
                                                 start=True, stop=True),
                             "pe" if c == 3 else None)

                yw = [("pe", g_z)] + ([("dve", dyr[i - 3])] if i >= 3 else [])
                zact.append(act(yw, lambda i=i: act_i(ybufs[i % 3][:], zps[i % 2][:],
                                                      AF.Tanh, scale=AA)))

                dyr.append(dve([("act", zact[i])],
                               lambda i=i: V.tensor_scalar_max(yrb[:], ybufs[i % 3][:],
                                                               0.0)))
                dve(None, lambda: V.tensor_scalar(wb[:], yrb[:], DA, CA,
                                                  ALU.mult, ALU.add))
                ow = [(f"dout{i % 3}", dout_i[i - 3])] if i >= 3 else [("act", a_yst)]
                dve(ow, lambda i=i: V.tensor_mul(outbufs[i % 3][:, 0:N], wb[:], yrb[:]))
                relu_d.append(dve(None, lambda i=i: V.tensor_scalar_max(
                    outbufs[i % 3][:, N:NT], y_st[:, i * T:(i + 1) * T], 0.0)))
                dout_i.append(op("sync", [("dve", relu_d[i])],
                                 lambda i=i, rs=rs: nc.sync.dma_start(
                                     out=out[rs, :], in_=outbufs[i % 3][:]),
                                 f"dout{i % 3}"))

            # ---- temporal ts rows; tt already staged in ttres ----
            # ts0 into zA (free after zact[6]), runs during zact[7]
            for c in range(4):
                g_ts0 = pe([("act", zact[6]), ("dmx", x_tsr)] if c == 0 else None,
                           lambda c=c: mm(zA[0:128, c * 512:(c + 1) * 512],
                                          tsL[:, 0:128], tsR[:, c * 512:(c + 1) * 512],
                                          start=True, stop=True),
                           "pe" if c == 3 else None)
            # ts1 into zB (free after zact[7])
            tn = TS - 128
            for c in range(4):
                g_ts1 = pe([("act", zact[7])] if c == 0 else None,
                           lambda c=c, tn=tn: mm(zB[0:tn, c * 512:(c + 1) * 512],
                                                 tsL[:, 128:TS],
                                                 tsR[:, c * 512:(c + 1) * 512],
                                                 start=True, stop=True),
                           "pe" if c == 3 else None)

            r = NCHUNK
            a_ts0 = act([("pe", g_ts0)],
                        lambda: act_i(ytb[0:128, :], zA[0:128, :], AF.Tanh))
            ow = [(f"dout{r % 3}", dout_i[r - 3]), ("act", a_ts0)]
            d_tsr0 = dve(ow, lambda r=r: V.tensor_scalar_max(
                outbufs[r % 3][0:128, 0:N], ytb[0:128, :], 0.0))
            relu_d.append(dve(None, lambda r=r: V.tensor_copy(
                outbufs[r % 3][0:128, N:NT], ttres[0][:])))
            dout_i.append(op("sync", [("dve", relu_d[r])],
                             lambda r=r: nc.sync.dma_start(
                                 out=out[NS:NS + 128, :], in_=outbufs[r % 3][0:128, :]),
                             f"dout{r % 3}"))

            r = NCHUNK + 1
            a_ts1 = act([("pe", g_ts1), ("dve", d_tsr0)],
                        lambda tn=tn: act_i(ytb[0:tn, :], zB[0:tn, :], AF.Tanh))
            ow = [(f"dout{r % 3}", dout_i[r - 3]), ("act", a_ts1)]
            dve(ow, lambda tn=tn, r=r: V.tensor_scalar_max(
                outbufs[r % 3][0:tn, 0:N], ytb[0:tn, :], 0.0))
            relu_d.append(dve(None, lambda tn=tn, r=r: V.tensor_copy(
                outbufs[r % 3][0:tn, N:NT], ttres[1][:])))
            dout_i.append(op("sync", [("dve", relu_d[r])],
                             lambda tn=tn, r=r: nc.sync.dma_start(
                                 out=out[NS + 128:ROWS, :], in_=outbufs[r % 3][0:tn, :]),
                             f"dout{r % 3}"))

        # ---------- emit ----------
        with nc.Block() as block:
            def make_body(engine_name):
                ops = plan[engine_name]

                def body(eng):
                    satisfied = {}
                    for waits, fn, inc in ops:
                        for sem_name, val in waits:
                            if val is not None and satisfied.get(sem_name, -1) < val:
                                eng.wait_ge(SEM[sem_name], val)
                                satisfied[sem_name] = val
                        ins = fn()
                        if inc is None:
                            continue
                        if inc in ("pe", "act", "dve"):
                            ins.then_inc(SEM[inc], 1)
                        else:
                            ins.then_inc(SEM[inc], 16)
                return body

            block.sync(make_body("sync"))
            block.tensor(make_body("tensor"))
            block.scalar(make_body("scalar"))
            block.vector(make_body("vector"))

    return nc


def _bf16(x):
    u = x.astype(np.float32).view(np.uint32)
    r = ((u >> 16) + ((u >> 15) & 1)).astype(np.uint32) << 16
    return r.view(np.float32)


def build_in_maps(spatial_nodes, temporal_nodes, W_ss1, W_ss2, w_st, b_st, w_ts, b_ts):
    import ml_dtypes
    f, h = np.float32, np.float16
    bf = ml_dtypes.bfloat16

    def stack96(a32):
        hi = _bf16(a32)
        lo = _bf16(a32 - hi)
        return np.ascontiguousarray(np.concatenate([hi, lo, hi], axis=0)).astype(bf)

    def stackW(w32):
        hi = _bf16(w32)
        lo = _bf16(w32 - hi)
        return np.ascontiguousarray(np.concatenate([hi, hi, lo], axis=0)).astype(bf)

    W_R = np.concatenate([W_ss2.T, W_ss1.T], axis=1).astype(f)
    W_L = np.concatenate([W_ss1.T, -W_ss2.T], axis=1).astype(f)
    W96_R = stackW(W_R)
    W96_L = stackW(W_L)
    stind = np.zeros((9, NCHUNK * T), dtype=h)
    for k in range(NCHUNK):
        stind[k + 1, k * T:(k + 1) * T] = 1.0
    in_maps = []
    for c in range(N_CORES):
        b, hh = divmod(c, 2)
        tmask = (np.arange(T)[None, :] >= (hh * TS + np.arange(TS))[:, None]).astype(h)
        spT = np.ascontiguousarray(spatial_nodes[b].T, dtype=f)
        tmT = np.ascontiguousarray(temporal_nodes[b].T, dtype=f)
        spTr = np.ascontiguousarray(spT[:, hh * NS:(hh + 1) * NS])
        tmTr = np.ascontiguousarray(tmT[:, hh * TS:(hh + 1) * TS])
        sp96 = stack96(spT)
        in_maps.append({
            "sp96": sp96, "W96_R": W96_R,
            "sp96r": np.ascontiguousarray(sp96[:, hh * NS:(hh + 1) * NS]),
            "W96_L": W96_L,
            "spT16": spT.astype(h), "spTr16": spTr.astype(h),
            "tmT16": tmT.astype(h), "tmTr16": tmTr.astype(h),
            "tmT32": tmT, "tmTr32": tmTr,
            "wst_a16": np.ascontiguousarray(w_st[:D, None], dtype=h),
            "wst_b16": np.ascontiguousarray(w_st[D:, None], dtype=h),
            "wts_a16": np.ascontiguousarray(w_ts[:D, None], dtype=h),
            "wts_b16": np.ascontiguousarray(w_ts[D:, None], dtype=h),
            "bst": np.asarray(b_st, dtype=f).reshape(1, 1),
            "bts": np.asarray(b_ts, dtype=f).reshape(1, 1),
            "ttmask16": tmask,
            "stind": stind,
        })
    return in_maps


def assemble(results):
    out = np.empty((B, NT, NT), np.float32)
    for c in range(N_CORES):
        b, h = divmod(c, 2)
        r = results[c]["out"].astype(np.float32)
        out[b, h * NS:(h + 1) * NS, :] = r[0:NS]
        out[b, N + h * TS: N + (h + 1) * TS, :] = r[NS:ROWS]
    return out


_NC = None


def kernel(**inputs):
    global _NC
    if _NC is None:
        _NC = build_program()
    in_maps = build_in_maps(**inputs)
    res = run_bass_kernel_spmd(_NC, in_maps, list(range(N_CORES)))
    return assemble(res.results)
